# revision 16
# baseline (speedup 1.0000x reference)
"""GPS layer (GraphGPS) Trainium2 kernel: ResGatedGraphConv + dense per-graph MHA + FFN.

Sharding: data-parallel over the 64 graphs (8 graphs / 4096 nodes per core).
Edges are sorted by destination and bucketed into 128-node windows per core;
message aggregation uses one-hot matmuls accumulating in PSUM per window.
q|v rows (keyed by random src) are fetched via window-batched indirect DMA
from a device-computed [N, 256] bf16 q|v table in DRAM. All matmuls run in
bf16 (fp32 PSUM accumulate); BN/residual chains stay fp32.
"""
import sys
sys.path.insert(0, '/opt/trn_rl_repo')
import numpy as np
import ml_dtypes
import concourse.bass as bass
import concourse.bacc as bacc
import concourse.tile as tile
from concourse import mybir
from concourse.bass_utils import run_bass_kernel_spmd
from concourse.masks import make_identity

N, E, D, H, B, S = 32768, 524288, 128, 4, 64, 512
HD = D // H          # 32
NC = 8               # cores
NPC = N // NC        # 4096 nodes per core
GPC = B // NC        # 8 graphs per core
WIN = 128            # dst window
NWIN = NPC // WIN    # 32 windows per core
EPS = 1e-5
F32 = mybir.dt.float32
BF16 = mybir.dt.bfloat16
I32 = mybir.dt.int32
BF = ml_dtypes.bfloat16


def _prep_edges(edge_index):
    src = np.asarray(edge_index[0], dtype=np.int64)
    dst = np.asarray(edge_index[1], dtype=np.int64)
    order = np.argsort(dst, kind='stable')
    ss, ds = src[order], dst[order]
    wid = ds // WIN                       # global window id, 0..255
    counts = np.bincount(wid, minlength=NC * NWIN)
    tpw = int(np.ceil(counts.max() / 128))   # tiles per window (uniform)
    cap = tpw * 128
    gsrc = np.zeros((NC * NWIN, cap), np.int32)
    ldst = np.full((NC * NWIN, cap), -1.0, np.float32)
    offs = np.zeros(NC * NWIN + 1, np.int64)
    np.cumsum(counts, out=offs[1:])
    for w in range(NC * NWIN):
        s, e = offs[w], offs[w + 1]
        n = e - s
        gsrc[w, :n] = ss[s:e]
        ldst[w, :n] = (ds[s:e] - w * WIN).astype(np.float32)
    # -> per-core [128, NWIN*tpw]: edge p of tile t of window w at
    # [p, w*tpw + t] (partition-major so one contiguous DMA loads all)
    gsrc = gsrc.reshape(NC, NWIN, tpw, 128).transpose(0, 3, 1, 2)
    ldst = ldst.reshape(NC, NWIN, tpw, 128).transpose(0, 3, 1, 2)
    gsrc = np.ascontiguousarray(gsrc.reshape(NC, 128, NWIN * tpw))
    ldst = np.ascontiguousarray(ldst.reshape(NC, 128, NWIN * tpw))
    return gsrc, ldst, tpw


def _build(nc, tpw):
    P = 128
    xT = nc.declare_dram_parameter("xT", [P, N], BF16, isOutput=False)
    xT_loc = nc.declare_dram_parameter("xT_loc", [P, NPC], BF16, isOutput=False)
    gsrcT = nc.declare_dram_parameter("gsrcT", [P, NWIN * tpw], I32, isOutput=False)
    ldstT = nc.declare_dram_parameter("ldstT", [P, NWIN * tpw], F32, isOutput=False)
    wqvT = nc.declare_dram_parameter("wqvT", [P, 2 * P], BF16, isOutput=False)
    WkT = nc.declare_dram_parameter("WkT", [P, P], BF16, isOutput=False)
    WresT = nc.declare_dram_parameter("WresT", [P, P], BF16, isOutput=False)
    WskipT = nc.declare_dram_parameter("WskipT", [P, P], BF16, isOutput=False)
    ipwT = nc.declare_dram_parameter("ipwT", [P, 3 * P], BF16, isOutput=False)
    opwT = nc.declare_dram_parameter("opwT", [P, P], BF16, isOutput=False)
    W1T = nc.declare_dram_parameter("W1T", [P, 2 * P], BF16, isOutput=False)
    W2Ta = nc.declare_dram_parameter("W2Ta", [P, P], BF16, isOutput=False)
    W2Tb = nc.declare_dram_parameter("W2Tb", [P, P], BF16, isOutput=False)
    qvb = nc.declare_dram_parameter("qvb", [P, 2 * P], F32, isOutput=False)
    # column vectors [128, k]: biases and folded-BN scale/shift
    cols = nc.declare_dram_parameter("cols", [P, 10], F32, isOutput=False)
    ipb = nc.declare_dram_parameter("ipb", [P, 3], F32, isOutput=False)
    b1c = nc.declare_dram_parameter("b1c", [P, 2], F32, isOutput=False)
    qv_dram = nc.dram_tensor("qv_table", [N, 2 * P], BF16)
    outT = nc.declare_dram_parameter("outT", [P, NPC], F32, isOutput=True)

    rsq = float(1.0 / np.sqrt(HD))
    with tile.TileContext(nc) as tc:
        import contextlib
        with contextlib.ExitStack() as es:
            one = es.enter_context(tc.tile_pool(name="one", bufs=1))
            sbA = es.enter_context(tc.tile_pool(name="sbA", bufs=2))
            sbG = es.enter_context(tc.tile_pool(name="sbG", bufs=2))
            sbO = es.enter_context(tc.tile_pool(name="sbO", bufs=2))
            sbT = es.enter_context(tc.tile_pool(name="sbT", bufs=3))
            at = es.enter_context(tc.tile_pool(name="at", bufs=2))
            sbE = es.enter_context(tc.tile_pool(name="sbE", bufs=1))
            ps5 = es.enter_context(tc.tile_pool(name="ps5", bufs=2, space="PSUM"))
            psT = es.enter_context(tc.tile_pool(name="psT", bufs=2, space="PSUM"))
            psP = es.enter_context(tc.tile_pool(name="psP", bufs=2, space="PSUM"))
            psAg = es.enter_context(tc.tile_pool(name="psAg", bufs=1, space="PSUM"))
            psC = es.enter_context(tc.tile_pool(name="psC", bufs=1, space="PSUM"))

            identb = one.tile([P, P], BF16)
            make_identity(nc, identb[:])
            identf = one.tile([P, P], F32)
            make_identity(nc, identf[:])
            wqv = one.tile([P, 2 * P], BF16); nc.sync.dma_start(out=wqv[:], in_=wqvT[:])
            wk = one.tile([P, P], BF16); nc.sync.dma_start(out=wk[:], in_=WkT[:])
            wres = one.tile([P, P], BF16); nc.sync.dma_start(out=wres[:], in_=WresT[:])
            wskip = one.tile([P, P], BF16); nc.sync.dma_start(out=wskip[:], in_=WskipT[:])
            wip = one.tile([P, 3 * P], BF16); nc.sync.dma_start(out=wip[:], in_=ipwT[:])
            wop = one.tile([P, P], BF16); nc.sync.dma_start(out=wop[:], in_=opwT[:])
            w1 = one.tile([P, 2 * P], BF16); nc.sync.dma_start(out=w1[:], in_=W1T[:])
            w2a = one.tile([P, P], BF16); nc.sync.dma_start(out=w2a[:], in_=W2Ta[:])
            w2b = one.tile([P, P], BF16); nc.sync.dma_start(out=w2b[:], in_=W2Tb[:])
            qvbv = one.tile([P, 2 * P], F32); nc.sync.dma_start(out=qvbv[:], in_=qvb[:])
            colv = one.tile([P, 10], F32); nc.sync.dma_start(out=colv[:], in_=cols[:])
            ipbv = one.tile([P, 3], F32); nc.sync.dma_start(out=ipbv[:], in_=ipb[:])
            b1v = one.tile([P, 2], F32); nc.sync.dma_start(out=b1v[:], in_=b1c[:])
            gsw = one.tile([P, NWIN * tpw], I32); nc.sync.dma_start(out=gsw[:], in_=gsrcT[:])
            ldw = one.tile([P, NWIN * tpw], F32); nc.sync.dma_start(out=ldw[:], in_=ldstT[:])
            iota_r = one.tile([P, P], I32)
            nc.gpsimd.iota(iota_r[:], pattern=[[1, P]], base=0, channel_multiplier=0)
            iota_f = one.tile([P, P], F32)
            nc.vector.tensor_copy(iota_f[:], iota_r[:])

            # ---- phase 1a: qv table [N, 256] bf16 -> DRAM ----
            # q rows carry bias (bq + bk) so the edge sigmoid needs no extra add
            for blk in range(8):           # xT in [128, 4096] chunks
                xc = sbA.tile([P, 4096], BF16, tag="xc")
                nc.sync.dma_start(out=xc[:], in_=xT[:, blk * 4096:(blk + 1) * 4096])
                for grp in range(4):       # 8 node-chunks of 128 per write
                    qvt = sbA.tile([P, 8 * 256], BF16, tag="qvt")
                    for j in range(0, 8, 2):
                        c = grp * 8 + j
                        pt = ps5.tile([P, 512], F32, tag="b512")
                        nc.tensor.matmul(pt[:, :256], lhsT=xc[:, c * P:(c + 1) * P],
                                         rhs=wqv[:], start=True, stop=True)
                        nc.tensor.matmul(pt[:, 256:], lhsT=xc[:, (c + 1) * P:(c + 2) * P],
                                         rhs=wqv[:], start=True, stop=True)
                        nc.vector.tensor_tensor(
                            out=qvt[:, j * 256:(j + 2) * 256].rearrange("p (u x) -> p u x", u=2),
                            in0=pt[:].rearrange("p (u x) -> p u x", u=2),
                            in1=qvbv[:, None, :].to_broadcast([P, 2, 256]),
                            op=mybir.AluOpType.add)
                    r0 = (blk * 4 + grp) * 1024
                    nc.sync.dma_start(
                        out=qv_dram[r0:r0 + 1024, :].rearrange("(q p) x -> p q x", p=P),
                        in_=qvt[:].rearrange("p (q x) -> p q x", q=8))

            # ---- phase 1b: local tables ----
            hin1f = one.tile([P, NPC], F32)   # dim-major relu(x@WresT+bres)
            hin1b = one.tile([P, NPC], BF16)
            skipf = one.tile([P, NPC], F32)
            ktab = one.tile([P, NWIN * P], BF16)  # node-major k per window
            hloc = one.tile([P, NPC], F32)
            xl = one.tile([P, NPC], BF16)
            nc.sync.dma_start(out=xl[:], in_=xT_loc[:])
            for c in range(8):
                sl = slice(c * 512, (c + 1) * 512)
                pr = ps5.tile([P, 512], F32, tag="b512")
                nc.tensor.matmul(pr[:], lhsT=wres[:], rhs=xl[:, sl], start=True, stop=True)
                nc.scalar.activation(hin1f[:, sl], pr[:],
                                     mybir.ActivationFunctionType.Relu,
                                     bias=colv[:, 0:1], scale=1.0)
                nc.vector.tensor_copy(hin1b[:, sl], hin1f[:, sl])
                pr2 = ps5.tile([P, 512], F32, tag="b512")
                nc.tensor.matmul(pr2[:], lhsT=wskip[:], rhs=xl[:, sl], start=True, stop=True)
                nc.scalar.activation(skipf[:, sl], pr2[:],
                                     mybir.ActivationFunctionType.Identity,
                                     bias=colv[:, 1:2], scale=1.0)
            for w in range(NWIN):
                pk = psP.tile([P, P], F32, tag="parg")
                nc.tensor.matmul(pk[:], lhsT=xl[:, w * P:(w + 1) * P], rhs=wk[:],
                                 start=True, stop=True)
                nc.vector.tensor_copy(ktab[:, w * P:(w + 1) * P], pk[:])

            qblk = one.tile([P, 4 * S], BF16)   # per-head-masked q, zeros elsewhere
            nc.vector.memset(qblk[:], 0.0)
            dens = one.tile([P, S], F32)        # head h softmax denom at partition h*32
            nc.vector.memset(dens[:], 1.0)

            # ---- phases 2+3 interleaved: 4 windows then 1 graph ----
            for g in range(GPC):
                for w in range(4 * g, 4 * g + 4):
                    msl = slice(w * tpw, (w + 1) * tpw)
                    gat = sbG.tile([P, tpw * 256], BF16, tag="gat")
                    nc.gpsimd.indirect_dma_start(
                        out=gat[:], out_offset=None, in_=qv_dram[:],
                        in_offset=bass.IndirectOffsetOnAxis(ap=gsw[:, msl], axis=0))
                    obig = sbO.tile([P, tpw * P], BF16, tag="obig")
                    nc.vector.tensor_tensor(
                        out=obig[:].rearrange("p (t n) -> p t n", t=tpw),
                        in0=ldw[:, msl, None].to_broadcast([P, tpw, P]),
                        in1=iota_f[:, None, :].to_broadcast([P, tpw, P]),
                        op=mybir.AluOpType.is_equal)
                    agg = psAg.tile([P, P], F32, tag="agg")
                    kwin = ktab[:, w * P:(w + 1) * P]
                    for t in range(tpw):
                        osl = obig[:, t * P:(t + 1) * P]
                        pot = psT.tile([P, P], BF16, tag="pot")
                        nc.tensor.transpose(out=pot[:], in_=osl, identity=identb[:])
                        ot = sbT.tile([P, P], BF16, tag="ot")
                        nc.vector.tensor_copy(ot[:], pot[:])
                        parg = psP.tile([P, P], F32, tag="parg")
                        nc.tensor.matmul(parg[:], lhsT=ot[:], rhs=kwin, start=True, stop=False)
                        nc.tensor.matmul(parg[:], lhsT=identb[:], rhs=gat[:, t * 256:t * 256 + P],
                                         start=False, stop=True)
                        sig = sbT.tile([P, P], BF16, tag="sig")
                        nc.scalar.activation(sig[:], parg[:],
                                             mybir.ActivationFunctionType.Sigmoid)
                        msg = sbT.tile([P, P], BF16, tag="msg")
                        nc.vector.tensor_mul(out=msg[:], in0=sig[:], in1=gat[:, t * 256 + P:(t + 1) * 256])
                        nc.tensor.matmul(agg[:], lhsT=osl, rhs=msg[:],
                                         start=(t == 0), stop=(t == tpw - 1))
                    asb = sbT.tile([P, P], F32, tag="asb")
                    nc.vector.tensor_copy(asb[:], agg[:])
                    paT = psP.tile([P, P], F32, tag="parg")
                    nc.tensor.transpose(out=paT[:], in_=asb[:], identity=identf[:])
                    wsl = slice(w * P, (w + 1) * P)
                    t1 = sbT.tile([P, P], F32, tag="t1")
                    nc.vector.tensor_add(out=t1[:], in0=paT[:], in1=skipf[:, wsl])
                    nc.vector.tensor_add(out=t1[:], in0=t1[:], in1=hin1f[:, wsl])
                    nc.vector.tensor_scalar(out=hloc[:, wsl], in0=t1[:],
                                            scalar1=colv[:, 2:3], scalar2=colv[:, 3:4],
                                            op0=mybir.AluOpType.mult,
                                            op1=mybir.AluOpType.add)

                # ---- graph g: dense MHA + FFN ----
                gs = slice(g * S, (g + 1) * S)
                hgb = hin1b[:, gs]
                pq = ps5.tile([P, S], F32, tag="b512")
                nc.tensor.matmul(pq[:], lhsT=wip[:, 0:P], rhs=hgb, start=True, stop=True)
                qtmp = at.tile([P, S], BF16, tag="qtmp")
                nc.scalar.activation(qtmp[:], pq[:],
                                     mybir.ActivationFunctionType.Identity,
                                     bias=ipbv[:, 0:1], scale=1.0)
                pk2 = ps5.tile([P, S], F32, tag="b512")
                nc.tensor.matmul(pk2[:], lhsT=wip[:, P:2 * P], rhs=hgb, start=True, stop=True)
                ksb = at.tile([P, S], BF16, tag="ksb")
                nc.scalar.activation(ksb[:], pk2[:],
                                     mybir.ActivationFunctionType.Identity,
                                     bias=ipbv[:, 1:2], scale=1.0)
                pv = ps5.tile([P, S], F32, tag="b512")
                nc.tensor.matmul(pv[:], lhsT=wip[:, 2 * P:3 * P], rhs=hgb, start=True, stop=True)
                vsb = at.tile([P, S], BF16, tag="vsb")
                nc.scalar.activation(vsb[:], pv[:],
                                     mybir.ActivationFunctionType.Identity,
                                     bias=ipbv[:, 2:3], scale=1.0)
                for h in range(H):
                    hp = slice(h * HD, (h + 1) * HD)
                    nc.vector.tensor_copy(qblk[hp, h * S:(h + 1) * S], qtmp[hp, :])
                # scores^T = k_chunk^T @ q_blk, exp -> esc (softmax via aug-ones AV)
                esc = sbE.tile([P, 16 * S], BF16, tag="esc")
                for c in range(4):
                    for h in range(H):
                        pS = ps5.tile([P, S], F32, tag="b512")
                        nc.tensor.matmul(pS[:], lhsT=ksb[:, c * P:(c + 1) * P],
                                         rhs=qblk[:, h * S:(h + 1) * S], start=True, stop=True)
                        nc.scalar.activation(esc[:, (c * 4 + h) * S:(c * 4 + h + 1) * S], pS[:],
                                             mybir.ActivationFunctionType.Exp, scale=rsq)
                vaugs = []
                for c in range(4):
                    vaug = at.tile([P, 4 * (HD + 1)], BF16, tag=f"vaug{c}")
                    pvT = psT.tile([P, P], BF16, tag="pot")
                    nc.tensor.transpose(out=pvT[:], in_=vsb[:, c * P:(c + 1) * P],
                                        identity=identb[:])
                    for h in range(H):
                        nc.vector.tensor_copy(vaug[:, h * (HD + 1):h * (HD + 1) + HD],
                                              pvT[:, h * HD:(h + 1) * HD])
                        nc.vector.memset(vaug[:, h * (HD + 1) + HD:(h + 1) * (HD + 1)], 1.0)
                    vaugs.append(vaug)
                ctxg = at.tile([P, S], BF16, tag="ctxg")
                ctxu = at.tile([P, S], F32, tag="ctxu")
                for h in range(H):
                    pc = psC.tile([HD + 1, S], F32, tag="pctx")
                    for c in range(4):
                        nc.tensor.matmul(pc[:], lhsT=vaugs[c][:, h * (HD + 1):(h + 1) * (HD + 1)],
                                         rhs=esc[:, (c * 4 + h) * S:(c * 4 + h + 1) * S],
                                         start=(c == 0), stop=(c == 3))
                    nc.vector.tensor_copy(dens[h * HD:h * HD + 1, :], pc[HD:HD + 1, :])
                    nc.vector.tensor_copy(ctxu[h * HD:(h + 1) * HD, :], pc[:HD, :])
                rden = at.tile([P, S], F32, tag="rden")
                nc.vector.reciprocal(rden[:], dens[:])
                denb = at.tile([P, S], F32, tag="denb")
                for h in range(H):
                    nc.gpsimd.partition_broadcast(denb[h * HD:(h + 1) * HD, :],
                                                  rden[h * HD:h * HD + 1, :])
                nc.vector.tensor_mul(out=ctxg[:], in0=ctxu[:], in1=denb[:])
                # out proj + BN1a combine with h_in1, then FFN + BN2
                pop = ps5.tile([P, S], F32, tag="b512")
                nc.tensor.matmul(pop[:], lhsT=wop[:], rhs=ctxg[:], start=True, stop=True)
                hat = at.tile([P, S], F32, tag="hat")
                nc.scalar.activation(hat[:], pop[:],
                                     mybir.ActivationFunctionType.Identity,
                                     bias=colv[:, 4:5], scale=1.0)
                nc.vector.tensor_add(out=hat[:], in0=hat[:], in1=hin1f[:, gs])
                nc.vector.tensor_scalar(out=hat[:], in0=hat[:],
                                        scalar1=colv[:, 5:6], scalar2=colv[:, 6:7],
                                        op0=mybir.AluOpType.mult,
                                        op1=mybir.AluOpType.add)
                nc.vector.tensor_add(out=hat[:], in0=hat[:], in1=hloc[:, gs])
                hatb = at.tile([P, S], BF16, tag="hatb")
                nc.vector.tensor_copy(hatb[:], hat[:])
                ff = []
                for cc in range(2):
                    pf = ps5.tile([P, S], F32, tag="b512")
                    nc.tensor.matmul(pf[:], lhsT=w1[:, cc * P:(cc + 1) * P], rhs=hatb[:],
                                     start=True, stop=True)
                    ffc = at.tile([P, S], BF16, tag=f"ff{cc}")
                    nc.scalar.activation(ffc[:], pf[:],
                                         mybir.ActivationFunctionType.Relu,
                                         bias=b1v[:, cc:cc + 1], scale=1.0)
                    ff.append(ffc)
                pf2 = ps5.tile([P, S], F32, tag="b512")
                nc.tensor.matmul(pf2[:], lhsT=w2a[:], rhs=ff[0][:], start=True, stop=False)
                nc.tensor.matmul(pf2[:], lhsT=w2b[:], rhs=ff[1][:], start=False, stop=True)
                ot2 = at.tile([P, S], F32, tag="ot2")
                nc.scalar.activation(ot2[:], pf2[:],
                                     mybir.ActivationFunctionType.Identity,
                                     bias=colv[:, 7:8], scale=1.0)
                nc.vector.tensor_add(out=ot2[:], in0=ot2[:], in1=hat[:])
                nc.vector.tensor_scalar(out=ot2[:], in0=ot2[:],
                                        scalar1=colv[:, 8:9], scalar2=colv[:, 9:10],
                                        op0=mybir.AluOpType.mult,
                                        op1=mybir.AluOpType.add)
                nc.sync.dma_start(out=outT[:, gs], in_=ot2[:])
    nc.compile()
    return nc


def kernel(x, edge_index, batch_ids, Wres, bres, Wk, bk, Wq, bq, Wv, bv,
           Wskip, bskip, g1l, b1l, g1a, b1a, in_proj_w, in_proj_b,
           out_proj_w, out_proj_b, W1, b1, W2, b2, g2, b2g):
    x = np.asarray(x, dtype=np.float32)
    gsrcT, ldstT, tpw = _prep_edges(np.asarray(edge_index))
    xT = np.ascontiguousarray(x.T.astype(BF))
    bnf = 1.0 / np.sqrt(1.0 + EPS)
    cols = np.zeros((128, 10), np.float32)
    cols[:, 0] = bres; cols[:, 1] = bskip
    cols[:, 2] = g1l * bnf; cols[:, 3] = b1l
    cols[:, 4] = out_proj_b
    cols[:, 5] = g1a * bnf; cols[:, 6] = b1a
    cols[:, 7] = b2; cols[:, 8] = g2 * bnf; cols[:, 9] = b2g
    qvb = np.tile(np.concatenate([np.asarray(bq) + np.asarray(bk),
                                  np.asarray(bv)]).astype(np.float32)[None, :], (128, 1))

    def bf(a):
        return np.ascontiguousarray(np.asarray(a, np.float32).astype(BF))

    common = dict(
        xT=xT,
        wqvT=bf(np.concatenate([np.asarray(Wq).T, np.asarray(Wv).T], axis=1)),
        WkT=bf(np.asarray(Wk).T), WresT=bf(np.asarray(Wres).T),
        WskipT=bf(np.asarray(Wskip).T),
        ipwT=bf(np.asarray(in_proj_w).T), opwT=bf(np.asarray(out_proj_w).T),
        W1T=bf(np.asarray(W1).T),
        W2Ta=bf(np.asarray(W2).T[:128]), W2Tb=bf(np.asarray(W2).T[128:]),
        qvb=np.ascontiguousarray(qvb), cols=cols,
        ipb=np.ascontiguousarray(np.asarray(in_proj_b, np.float32).reshape(3, 128).T),
        b1c=np.ascontiguousarray(np.asarray(b1, np.float32).reshape(2, 128).T),
    )
    in_maps = []
    for c in range(NC):
        m = dict(common)
        m["xT_loc"] = np.ascontiguousarray(xT[:, c * NPC:(c + 1) * NPC])
        m["gsrcT"] = gsrcT[c]
        m["ldstT"] = ldstT[c]
        in_maps.append(m)

    nc = bacc.Bacc("TRN2", target_bir_lowering=False, debug=False, num_devices=NC)
    _build(nc, tpw)
    res = run_bass_kernel_spmd(nc, in_maps, list(range(NC)))
    if getattr(res, "exec_time_ns", None):
        print(f"HW exec time: {res.exec_time_ns} ns")
    out = np.empty((N, D), np.float32)
    for c in range(NC):
        out[c * NPC:(c + 1) * NPC] = res.results[c]["outT"].T
    return out
